# revision 11
# baseline (speedup 1.0000x reference)
"""BertSelfAttention Trainium2 kernel.

Shapes: hidden_states [S=1024, B=4, D=1024], H=16 heads of DH=64.
Sharding: 2 heads per core (8 cores). Each core receives the full hidden
states (pre-transposed + bf16-cast on host) and a 128-row slice of each
projection weight, computes the full attention chain for its two heads with
no cross-core communication.

Device-side layout tricks:
  - scores are computed transposed (scoresT[u, t] = q_t . k_u) so the
    additive attention mask (per key position u) is a per-partition bias
    that fuses into the Exp activation: probsT = exp(scores/8 + mask).
    Both heads' K=64 score matmuls are row-tiled into the PE array
    concurrently (row_grp 0 / 64) and write one shared [128, 2, 512] PSUM
    tile so the pair always issues back-to-back.
  - AV runs in out=[t, j] orientation: lhsT = probsT chunk (K=128, M=128
    query positions), rhs = V tile with a prepended ones-column (N=65), so
    ctx rows land on partitions and the softmax denominator lands in
    column 0. Normalization is then a per-partition reciprocal + scalar
    multiply on the vector engine (no cross-partition broadcast needed).
  - software pipeline: scores/exp paced by the scalar engine; AV of the
    previous batch, V projection and Q/K projection of later batches woven
    into the PE stream by a cost-paced queue so every batch window carries
    an even PE load.
"""

import os
import numpy as np
import ml_dtypes

S, B, D, H = 1024, 4, 1024, 16
DH = D // H          # 64
NCORES = 8
HPC = H // NCORES    # heads per core = 2
P = 128              # partitions / d-tile / t-tile
DCH = D // P         # 8 contraction tiles
BS = B * S           # 4096 flattened (b, s)
CH = 512             # matmul free-dim chunk (fp32 rhs limit)
TT = S // P          # 8 query tiles per batch

_compiled_nc = None
last_exec_time_ns = None
last_results = None


def _build():
    import concourse.bacc as bacc
    import concourse.mybir as mybir
    import concourse.tile as tile
    from contextlib import ExitStack

    f32 = mybir.dt.float32
    bf16 = mybir.dt.bfloat16
    AF = mybir.ActivationFunctionType

    nc = bacc.Bacc("TRN2", target_bir_lowering=False, debug=False,
                   num_devices=NCORES)

    hT_d = nc.dram_tensor("hT", [D, BS], bf16, kind="ExternalInput")
    # weights pre-swizzled on host to [p, dc, m] so the DMA moves 2KB lines
    wq_d = nc.dram_tensor("wq", [P, DCH * P], bf16, kind="ExternalInput")
    wk_d = nc.dram_tensor("wk", [P, DCH * P], bf16, kind="ExternalInput")
    wv_d = nc.dram_tensor("wv", [P, DCH * P], bf16, kind="ExternalInput")
    # packed per-partition constants: [bq | bk | bvb(128) | maskT(8*4)]
    misc_d = nc.dram_tensor("misc", [P, 2 + P + DCH * B], f32,
                            kind="ExternalInput")
    # out[b, p, hl, tt, j]: query position s = tt*128 + p, feature = hl*64+j
    out_d = nc.dram_tensor("out", [B, P, HPC, TT, DH], f32,
                           kind="ExternalOutput")

    with tile.TileContext(nc) as tc, ExitStack() as ctx:
        persist = ctx.enter_context(tc.tile_pool(name="persist", bufs=1))
        probs_pool = ctx.enter_context(tc.tile_pool(name="probs", bufs=36))
        small = ctx.enter_context(tc.tile_pool(name="small", bufs=4))
        o_pool = ctx.enter_context(tc.tile_pool(name="outp", bufs=2))
        # PSUM budget (8 banks): scores 2x[128,2,512] (4) + proj 2 + av 2
        ps_sc = ctx.enter_context(tc.tile_pool(name="ps_sc", bufs=2, space="PSUM"))
        ps_mm = ctx.enter_context(tc.tile_pool(name="ps_mm", bufs=2, space="PSUM"))
        ps_av = ctx.enter_context(tc.tile_pool(name="ps_av", bufs=2, space="PSUM"))

        # ---- persistent SBUF tensors ----
        hT_sb = persist.tile([P, DCH, BS], bf16)        # hidden^T, d-tiled
        wq_sb = persist.tile([P, DCH, P], bf16)
        wk_sb = persist.tile([P, DCH, P], bf16)
        wv_sb = persist.tile([P, DCH, P], bf16)
        misc_sb = persist.tile([P, 2 + P + DCH * B], f32)
        qT_sb = persist.tile([P, BS], bf16)             # Q^T [i, t]
        kT_sb = persist.tile([P, BS], bf16)             # K^T [i, t]
        # V in [t, j] layout + ones column per head: [t-part, t-tile, head, DH+1]
        v_sb = persist.tile([P, BS // P, HPC, DH + 1], bf16)
        dummy_sb = persist.tile([P, CH], bf16)

        bq_sb = misc_sb[:, 0:1]
        bk_sb = misc_sb[:, 1:2]
        bvb_sb = misc_sb[:, 2:2 + P]

        def mask_bias(uc, bi):
            c = 2 + P + uc * B + bi
            return misc_sb[:, c:c + 1]

        # ---- input DMAs ----
        # wq/wk first (contiguous, host-swizzled), then batch 0's hT pieces
        # alternating between the SP and ACT HWDGE queues, then wv/misc,
        # batch 1 split across both queues, batches 2-3 on SP.
        hT_re = hT_d.ap().rearrange("(dc p) t -> p dc t", p=P)

        def hT_piece(q, dc, eng):
            qsl = slice(q * S, (q + 1) * S)
            eng.dma_start(hT_sb[:, dc, qsl], hT_re[:, dc, qsl])

        nc.gpsimd.memset(dummy_sb[:], 0.0)
        nc.sync.dma_start(wq_sb[:], wq_d.ap())
        nc.scalar.dma_start(wk_sb[:], wk_d.ap())
        hT_piece(0, 0, nc.sync)
        hT_piece(0, 1, nc.scalar)
        nc.scalar.dma_start(wv_sb[:], wv_d.ap())
        nc.scalar.dma_start(misc_sb[:], misc_d.ap())
        for dc in range(2, DCH):
            hT_piece(0, dc, nc.sync if dc % 2 == 0 else nc.scalar)
        for dc in range(DCH):
            hT_piece(1, dc, nc.sync if dc % 2 == 0 else nc.scalar)
        # batches 2-3 via the software DGE on the otherwise-idle gpsimd
        # queue so their issue slices stay off the SP/ACT streams.
        for q in range(2, B):
            for dc in range(DCH):
                hT_piece(q, dc, nc.gpsimd)

        nc.vector.memset(v_sb[:, :, :, 0:1], 1.0)

        # HAM warmup: dead matmuls ramp the PE clock while inputs load.
        for _ in range(6):
            d_ps = ps_av.tile([P, CH], f32, tag="av", name="d_ps")
            nc.tensor.matmul(d_ps[:], dummy_sb[:, 0:P], dummy_sb[:],
                             start=True, stop=True)

        scale = 1.0 / float(np.sqrt(DH))

        # ---- thunks for the 128-mode PE work queue (cost, fn) ----
        def emit_qk_chunk(w_sb, b_sb, dst, ci):
            sl = slice(ci * CH, (ci + 1) * CH)
            qk_ps = ps_mm.tile([P, CH], f32, tag="mm", name="qk_ps")
            for dc in range(DCH):
                nc.tensor.matmul(
                    qk_ps[:], w_sb[:, dc, :], hT_sb[:, dc, sl],
                    start=(dc == 0), stop=(dc == DCH - 1))
            nc.vector.tensor_scalar_add(dst[:, sl], qk_ps[:], b_sb[:])

        def emit_v_tile(tt):
            tsl = slice(tt * P, (tt + 1) * P)
            v_ps = ps_mm.tile([P, CH], f32, tag="mm", name="v_ps")
            for dc in range(DCH):
                nc.tensor.matmul(
                    v_ps[:, 0:P], hT_sb[:, dc, tsl], wv_sb[:, dc, :],
                    start=(dc == 0), stop=(dc == DCH - 1))
            nc.vector.tensor_add(
                v_sb[:, tt, 0:HPC, 1:DH + 1],
                v_ps[:, 0:P].rearrange("p (h j) -> p h j", j=DH),
                bvb_sb[:].rearrange("p (h j) -> p h j", j=DH))

        def emit_av_tile(bi, hl, tt, pps, o_sb, pool=None, mul_on_act=False):
            # ctx^T tile: out[t, 0] = softmax denominator, out[t, 1:] = ctx
            p_ = pool or ps_av
            av_ps = p_.tile([P, CH], f32, tag="mm" if p_ is ps_mm else "av",
                            name="av_ps")
            av = av_ps[:, 0:DH + 1]
            c2, t0 = tt // 4, (tt % 4) * P
            for uc in range(DCH):
                nc.tensor.matmul(
                    av, pps[uc][c2][:, hl, t0:t0 + P],
                    v_sb[:, bi * TT + uc, hl, :],
                    start=(uc == 0), stop=(uc == DCH - 1))
            rcp = small.tile([P, 1], f32, name="rcp")
            nc.vector.reciprocal_approx_fast(rcp[:], av_ps[:, 0:1])
            if mul_on_act:
                nc.scalar.activation(o_sb[:, hl, tt, :], av_ps[:, 1:DH + 1],
                                     AF.Copy, scale=rcp[:])
            else:
                nc.vector.tensor_scalar_mul(
                    o_sb[:, hl, tt, :], av_ps[:, 1:DH + 1], rcp[:])

        def qk_chunk_subthunks(w_sb, b_sb, dst, ci, nsub):
            st = {}
            step = DCH // nsub

            def sub(lo):
                def fn():
                    sl = slice(ci * CH, (ci + 1) * CH)
                    if lo == 0:
                        st["ps"] = ps_mm.tile([P, CH], f32, tag="mm",
                                              name="qk_ps")
                    for dc in range(lo, lo + step):
                        nc.tensor.matmul(
                            st["ps"][:], w_sb[:, dc, :], hT_sb[:, dc, sl],
                            start=(dc == 0), stop=(dc == DCH - 1))
                    if lo + step == DCH:
                        nc.vector.tensor_scalar_add(dst[:, sl], st["ps"][:],
                                                    b_sb[:])
                return (1.72 / nsub, fn)
            return [sub(i * step) for i in range(nsub)]

        def qk_thunks(bi, nsub=1):
            return [th for (w_sb_, b_sb_, dst_) in ((wq_sb, bq_sb, qT_sb),
                                                    (wk_sb, bk_sb, kT_sb))
                    for ci in (2 * bi, 2 * bi + 1)
                    for th in qk_chunk_subthunks(w_sb_, b_sb_, dst_, ci, nsub)]

        def v_thunks(bi):
            return [(0.45, lambda t=tt: emit_v_tile(t))
                    for tt in range(TT * bi, TT * bi + TT)]

        def av_thunks(bi, pps, o_sb, pools=(None,), order="tt",
                      mul_on_act=False):
            idx = ([(t, h) for t in range(TT) for h in range(HPC)]
                   if order == "tt" else
                   [(t, h) for h in range(HPC) for t in range(TT)])
            return [(0.33, lambda h=hl, t=tt, p=pools[i % len(pools)]:
                     emit_av_tile(bi, h, t, pps, o_sb, p, mul_on_act))
                    for i, (tt, hl) in enumerate(idx)]

        def interleave(a, b):
            out, ia, ib = [], 0, 0
            while ia < len(a) or ib < len(b):
                if ia < len(a):
                    out.append(a[ia]); ia += 1
                if ib < len(b):
                    out.append(b[ib]); ib += 1
            return out

        # ---- prologue: batch 0's Q/K, dc-major so the four PSUM
        # accumulation groups chase the arriving hT pieces concurrently.
        pro_specs = [(wq_sb, bq_sb, qT_sb, 0), (wq_sb, bq_sb, qT_sb, 1),
                     (wk_sb, bk_sb, kT_sb, 0), (wk_sb, bk_sb, kT_sb, 1)]
        pro_tiles = [ps_sc.tile([P, HPC, CH], f32, tag="sc", name="pro_ps")
                     for _ in range(2)]
        for dc in range(DCH):
            for g, (w_sb_, b_sb_, dst_, ci) in enumerate(pro_specs):
                nc.tensor.matmul(
                    pro_tiles[g // 2][:, g % 2, :], w_sb_[:, dc, :],
                    hT_sb[:, dc, ci * CH:(ci + 1) * CH],
                    start=(dc == 0), stop=(dc == DCH - 1))
        # c0 adds on DVE, c1 adds on ACT: batch 0's first score pair only
        # needs the c0 halves, so both engines converge on it fast.
        for g, (w_sb_, b_sb_, dst_, ci) in enumerate(pro_specs):
            osl = slice(ci * CH, (ci + 1) * CH)
            if ci % 2 == 0:
                nc.vector.tensor_scalar_add(
                    dst_[:, osl], pro_tiles[g // 2][:, g % 2, :], b_sb_[:])
            else:
                nc.scalar.activation(dst_[:, osl],
                                     pro_tiles[g // 2][:, g % 2, :],
                                     AF.Identity, bias=b_sb_[:])

        # ---- per-window PE work queues (flexible 128-mode thunks).
        # Q/K of later batches are pulled into earlier, ACT-slack windows;
        # thunks gated by late DMA arrivals sit at the tail of a window.
        prev = None          # (bi, pps, o_sb) of previous batch
        all_pps = []
        for bi in range(B):
            pps = []         # pps[uc][c2] = [128, 2, 512] bf16 probs tile
            o_sb = o_pool.tile([P, HPC, TT, DH], f32, name="o_sb")
            if bi == 0:
                queue = interleave(v_thunks(0), qk_thunks(1))
            elif bi < B - 1:
                queue = interleave(av_thunks(prev[0], prev[1], prev[2]),
                                   v_thunks(bi) + qk_thunks(bi + 1, nsub=4))
            else:
                queue = interleave(av_thunks(prev[0], prev[1], prev[2]),
                                   v_thunks(bi))
            total = sum(c for c, _ in queue)
            spent = 0.0
            for uc in range(DCH):
                usl = slice(bi * S + uc * P, bi * S + (uc + 1) * P)
                cpps = []
                for c2 in range(2):
                    qsl = slice(bi * S + c2 * CH, bi * S + (c2 + 1) * CH)
                    sc = ps_sc.tile([P, HPC, CH], f32, tag="sc", name="sc_ps")
                    for hl in range(HPC):
                        hsl = slice(hl * DH, (hl + 1) * DH)
                        nc.tensor.matmul(
                            sc[:, hl, :], kT_sb[hsl, usl], qT_sb[hsl, qsl],
                            start=True, stop=True)
                    pp = probs_pool.tile([P, HPC, CH], bf16, name="pp")
                    nc.scalar.activation(
                        pp[:], sc[:], AF.Exp,
                        bias=mask_bias(uc, bi), scale=scale)
                    cpps.append(pp)
                pps.append(cpps)
                target = total * (uc + 1) / DCH
                while queue and (spent < target or uc == DCH - 1):
                    c, th = queue.pop(0)
                    th()
                    spent += c
            if prev is not None:
                nc.sync.dma_start(out_d.ap()[prev[0]], prev[2][:])
            prev = (bi, pps, o_sb)
        # epilogue: last batch's attention output, head-major with the two
        # free PSUM pools alternating so consecutive AV groups double-buffer;
        # each head's half-DMA starts as soon as its groups are normalized.
        epi = av_thunks(prev[0], prev[1], prev[2], pools=(ps_av, ps_mm),
                        order="hl", mul_on_act=True)
        for i, (c, th) in enumerate(epi):
            th()
            if i == TT - 1:
                nc.sync.dma_start(out_d.ap()[prev[0]][:, 0], prev[2][:, 0])
        nc.sync.dma_start(out_d.ap()[prev[0]][:, 1], prev[2][:, 1])

    nc.compile()
    return nc


def _get_nc():
    global _compiled_nc
    if _compiled_nc is None:
        _compiled_nc = _build()
    return _compiled_nc


def prepare_in_maps(hidden_states, attention_mask, Wq, bq, Wk, bk, Wv, bv):
    bf16 = ml_dtypes.bfloat16

    hs = np.asarray(hidden_states, dtype=np.float32)            # [S, B, D]
    hT = np.ascontiguousarray(hs.transpose(2, 1, 0).reshape(D, BS)).astype(bf16)
    maskT = np.ascontiguousarray(
        np.asarray(attention_mask, dtype=np.float32).reshape(B, S).T)
    Wq = np.asarray(Wq, dtype=np.float32)
    Wk = np.asarray(Wk, dtype=np.float32)
    Wv = np.asarray(Wv, dtype=np.float32)
    bq = np.asarray(bq, dtype=np.float32)
    bk = np.asarray(bk, dtype=np.float32)
    bv = np.asarray(bv, dtype=np.float32)

    def swizzle(w, sl):
        # [p, dc, m] layout so the device DMA reads 2KB-contiguous lines
        wT = w[sl, :].T.reshape(DCH, P, P).transpose(1, 0, 2)
        return np.ascontiguousarray(wT.reshape(P, DCH * P)).astype(bf16)

    # maskT packed as [p, uc, b] -> [128, 32]
    mask_pk = maskT.reshape(DCH, P, B).transpose(1, 0, 2).reshape(P, DCH * B)
    in_maps = []
    for c in range(NCORES):
        sl = slice(P * c, P * (c + 1))
        misc = np.empty((P, 2 + P + DCH * B), dtype=np.float32)
        misc[:, 0] = bq[sl]
        misc[:, 1] = bk[sl]
        misc[:, 2:2 + P] = np.broadcast_to(bv[sl][None, :], (P, P))
        misc[:, 2 + P:] = mask_pk
        in_maps.append({
            "hT": hT,
            "wq": swizzle(Wq, sl),
            "wk": swizzle(Wk, sl),
            "wv": swizzle(Wv, sl),
            "misc": misc,
        })
    return in_maps


def kernel(hidden_states, attention_mask, Wq, bq, Wk, bk, Wv, bv):
    global last_exec_time_ns, last_results
    from concourse.bass_utils import run_bass_kernel_spmd

    nc = _get_nc()
    in_maps = prepare_in_maps(hidden_states, attention_mask,
                              Wq, bq, Wk, bk, Wv, bv)

    trace = bool(int(os.environ.get("KERNEL_TRACE", "0")))
    tmpdir = os.environ.get("KERNEL_TRACE_DIR") or None
    res = run_bass_kernel_spmd(nc, in_maps, core_ids=list(range(NCORES)),
                               trace=trace, tmpdir=tmpdir)
    last_exec_time_ns = res.exec_time_ns
    last_results = res

    # gather: per-core out [B, P, HPC, TT, DH] -> full [S, B, D]
    outs = np.stack([np.asarray(res.results[c]["out"]) for c in range(NCORES)],
                    axis=0)                             # [C, B, p, hl, tt, j]
    # s = tt*128 + p ; d = c*128 + hl*64 + j
    full = outs.transpose(4, 2, 1, 0, 3, 5).reshape(S, B, D)
    return np.ascontiguousarray(full.astype(np.float32))


# revision 12
# speedup vs baseline: 1.0825x; 1.0825x over previous
"""BertSelfAttention Trainium2 kernel.

Shapes: hidden_states [S=1024, B=4, D=1024], H=16 heads of DH=64.
Sharding: 2 heads per core (8 cores). Each core receives the full hidden
states (pre-transposed + bf16-cast on host) and a 128-row slice of each
projection weight, computes the full attention chain for its two heads with
no cross-core communication.

Device-side layout tricks:
  - scores are computed transposed (scoresT[u, t] = q_t . k_u) so the
    additive attention mask (per key position u) is a per-partition bias
    that fuses into the Exp activation: probsT = exp(scores/8 + mask).
    Both heads' K=64 score matmuls are row-tiled into the PE array
    concurrently (row_grp 0 / 64) and write one shared [128, 2, 512] PSUM
    tile so the pair always issues back-to-back.
  - AV runs in out=[t, j] orientation: lhsT = probsT chunk (K=128, M=128
    query positions), rhs = V tile with a prepended ones-column (N=65), so
    ctx rows land on partitions and the softmax denominator lands in
    column 0. Normalization is then a per-partition reciprocal + scalar
    multiply on the vector engine (no cross-partition broadcast needed).
  - software pipeline: scores/exp paced by the scalar engine; AV of the
    previous batch, V projection and Q/K projection of later batches woven
    into the PE stream by a cost-paced queue so every batch window carries
    an even PE load.
"""

import os
import numpy as np
import ml_dtypes

S, B, D, H = 1024, 4, 1024, 16
DH = D // H          # 64
NCORES = 8
HPC = H // NCORES    # heads per core = 2
P = 128              # partitions / d-tile / t-tile
DCH = D // P         # 8 contraction tiles
BS = B * S           # 4096 flattened (b, s)
CH = 512             # matmul free-dim chunk (fp32 rhs limit)
TT = S // P          # 8 query tiles per batch

_compiled_nc = None
last_exec_time_ns = None
last_results = None


def _build():
    import concourse.bacc as bacc
    import concourse.mybir as mybir
    import concourse.tile as tile
    from contextlib import ExitStack

    f32 = mybir.dt.float32
    bf16 = mybir.dt.bfloat16
    AF = mybir.ActivationFunctionType

    nc = bacc.Bacc("TRN2", target_bir_lowering=False, debug=False,
                   num_devices=NCORES)

    hT_d = nc.dram_tensor("hT", [D, BS], bf16, kind="ExternalInput")
    # weights pre-swizzled on host to [p, dc, m] so the DMA moves 2KB lines
    wq_d = nc.dram_tensor("wq", [P, DCH * P], bf16, kind="ExternalInput")
    wk_d = nc.dram_tensor("wk", [P, DCH * P], bf16, kind="ExternalInput")
    wv_d = nc.dram_tensor("wv", [P, DCH * P], bf16, kind="ExternalInput")
    # packed per-partition constants: [bq | bk | bvb(128) | maskT(8*4)]
    misc_d = nc.dram_tensor("misc", [P, 2 + P + DCH * B], f32,
                            kind="ExternalInput")
    # out[b, p, hl, tt, j]: query position s = tt*128 + p, feature = hl*64+j
    out_d = nc.dram_tensor("out", [B, P, HPC, TT, DH], f32,
                           kind="ExternalOutput")

    with tile.TileContext(nc) as tc, ExitStack() as ctx:
        persist = ctx.enter_context(tc.tile_pool(name="persist", bufs=1))
        probs_pool = ctx.enter_context(tc.tile_pool(name="probs", bufs=36))
        small = ctx.enter_context(tc.tile_pool(name="small", bufs=4))
        o_pool = ctx.enter_context(tc.tile_pool(name="outp", bufs=2))
        # PSUM budget (8 banks): scores 2x[128,2,512] (4) + proj 2 + av 2
        ps_sc = ctx.enter_context(tc.tile_pool(name="ps_sc", bufs=2, space="PSUM"))
        ps_mm = ctx.enter_context(tc.tile_pool(name="ps_mm", bufs=2, space="PSUM"))
        ps_av = ctx.enter_context(tc.tile_pool(name="ps_av", bufs=2, space="PSUM"))

        # ---- persistent SBUF tensors ----
        hT_sb = persist.tile([P, DCH, BS], bf16)        # hidden^T, d-tiled
        wq_sb = persist.tile([P, DCH, P], bf16)
        wk_sb = persist.tile([P, DCH, P], bf16)
        wv_sb = persist.tile([P, DCH, P], bf16)
        misc_sb = persist.tile([P, 2 + P + DCH * B], f32)
        qT_sb = persist.tile([P, BS], bf16)             # Q^T [i, t]
        kT_sb = persist.tile([P, BS], bf16)             # K^T [i, t]
        # V in [t, j] layout + ones column per head: [t-part, t-tile, head, DH+1]
        v_sb = persist.tile([P, BS // P, HPC, DH + 1], bf16)
        dummy_sb = persist.tile([P, CH], bf16)

        bq_sb = misc_sb[:, 0:1]
        bk_sb = misc_sb[:, 1:2]
        bvb_sb = misc_sb[:, 2:2 + P]

        def mask_bias(uc, bi):
            c = 2 + P + uc * B + bi
            return misc_sb[:, c:c + 1]

        # ---- input DMAs ----
        # wq/wk first (contiguous, host-swizzled), then batch 0's hT pieces
        # alternating between the SP and ACT HWDGE queues, then wv/misc,
        # batch 1 split across both queues, batches 2-3 on SP.
        hT_re = hT_d.ap().rearrange("(dc p) t -> p dc t", p=P)

        def hT_piece(q, dc, eng):
            qsl = slice(q * S, (q + 1) * S)
            eng.dma_start(hT_sb[:, dc, qsl], hT_re[:, dc, qsl])

        nc.gpsimd.memset(dummy_sb[:], 0.0)
        nc.sync.dma_start(wq_sb[:], wq_d.ap())
        nc.scalar.dma_start(wk_sb[:], wk_d.ap())
        hT_piece(0, 0, nc.sync)
        hT_piece(0, 1, nc.scalar)
        nc.scalar.dma_start(wv_sb[:], wv_d.ap())
        nc.scalar.dma_start(misc_sb[:], misc_d.ap())
        for dc in range(2, DCH):
            hT_piece(0, dc, nc.sync if dc % 2 == 0 else nc.scalar)
        for dc in range(DCH):
            hT_piece(1, dc, nc.sync if dc % 2 == 0 else nc.scalar)
        # batches 2-3 on SP in two bulk DMAs each, naturally serialized
        # behind batches 0-1 so they don't steal HBM bandwidth early.
        for q in range(2, B):
            for half in range(2):
                dsl = slice(half * 4, half * 4 + 4)
                qsl = slice(q * S, (q + 1) * S)
                nc.sync.dma_start(hT_sb[:, dsl, qsl], hT_re[:, dsl, qsl])

        nc.vector.memset(v_sb[:, :, :, 0:1], 1.0)

        # HAM warmup: dead matmuls ramp the PE clock while inputs load.
        for _ in range(6):
            d_ps = ps_av.tile([P, CH], f32, tag="av", name="d_ps")
            nc.tensor.matmul(d_ps[:], dummy_sb[:, 0:P], dummy_sb[:],
                             start=True, stop=True)

        scale = 1.0 / float(np.sqrt(DH))

        # ---- thunks for the 128-mode PE work queue (cost, fn) ----
        def emit_qk_chunk(w_sb, b_sb, dst, ci):
            sl = slice(ci * CH, (ci + 1) * CH)
            qk_ps = ps_mm.tile([P, CH], f32, tag="mm", name="qk_ps")
            for dc in range(DCH):
                nc.tensor.matmul(
                    qk_ps[:], w_sb[:, dc, :], hT_sb[:, dc, sl],
                    start=(dc == 0), stop=(dc == DCH - 1))
            nc.vector.tensor_scalar_add(dst[:, sl], qk_ps[:], b_sb[:])

        def emit_v_tile(tt):
            tsl = slice(tt * P, (tt + 1) * P)
            v_ps = ps_mm.tile([P, CH], f32, tag="mm", name="v_ps")
            for dc in range(DCH):
                nc.tensor.matmul(
                    v_ps[:, 0:P], hT_sb[:, dc, tsl], wv_sb[:, dc, :],
                    start=(dc == 0), stop=(dc == DCH - 1))
            nc.vector.tensor_add(
                v_sb[:, tt, 0:HPC, 1:DH + 1],
                v_ps[:, 0:P].rearrange("p (h j) -> p h j", j=DH),
                bvb_sb[:].rearrange("p (h j) -> p h j", j=DH))

        def emit_av_tile(bi, hl, tt, pps, o_sb, pool=None, mul_on_act=False):
            # ctx^T tile: out[t, 0] = softmax denominator, out[t, 1:] = ctx
            p_ = pool or ps_av
            av_ps = p_.tile([P, CH], f32, tag="mm" if p_ is ps_mm else "av",
                            name="av_ps")
            av = av_ps[:, 0:DH + 1]
            c2, t0 = tt // 4, (tt % 4) * P
            for uc in range(DCH):
                nc.tensor.matmul(
                    av, pps[uc][c2][:, hl, t0:t0 + P],
                    v_sb[:, bi * TT + uc, hl, :],
                    start=(uc == 0), stop=(uc == DCH - 1))
            rcp = small.tile([P, 1], f32, name="rcp")
            nc.vector.reciprocal_approx_fast(rcp[:], av_ps[:, 0:1])
            if mul_on_act:
                nc.scalar.activation(o_sb[:, hl, tt, :], av_ps[:, 1:DH + 1],
                                     AF.Copy, scale=rcp[:])
            else:
                nc.vector.tensor_scalar_mul(
                    o_sb[:, hl, tt, :], av_ps[:, 1:DH + 1], rcp[:])

        def qk_chunk_subthunks(w_sb, b_sb, dst, ci, nsub):
            st = {}
            step = DCH // nsub

            def sub(lo):
                def fn():
                    sl = slice(ci * CH, (ci + 1) * CH)
                    if lo == 0:
                        st["ps"] = ps_mm.tile([P, CH], f32, tag="mm",
                                              name="qk_ps")
                    for dc in range(lo, lo + step):
                        nc.tensor.matmul(
                            st["ps"][:], w_sb[:, dc, :], hT_sb[:, dc, sl],
                            start=(dc == 0), stop=(dc == DCH - 1))
                    if lo + step == DCH:
                        nc.vector.tensor_scalar_add(dst[:, sl], st["ps"][:],
                                                    b_sb[:])
                return (1.72 / nsub, fn)
            return [sub(i * step) for i in range(nsub)]

        def qk_thunks(bi, nsub=1):
            return [th for (w_sb_, b_sb_, dst_) in ((wq_sb, bq_sb, qT_sb),
                                                    (wk_sb, bk_sb, kT_sb))
                    for ci in (2 * bi, 2 * bi + 1)
                    for th in qk_chunk_subthunks(w_sb_, b_sb_, dst_, ci, nsub)]

        def v_thunks(bi):
            return [(0.45, lambda t=tt: emit_v_tile(t))
                    for tt in range(TT * bi, TT * bi + TT)]

        def av_thunks(bi, pps, o_sb, pools=(None,), order="tt",
                      mul_on_act=False):
            idx = ([(t, h) for t in range(TT) for h in range(HPC)]
                   if order == "tt" else
                   [(t, h) for h in range(HPC) for t in range(TT)])
            return [(0.33, lambda h=hl, t=tt, p=pools[i % len(pools)]:
                     emit_av_tile(bi, h, t, pps, o_sb, p, mul_on_act))
                    for i, (tt, hl) in enumerate(idx)]

        def interleave(a, b):
            out, ia, ib = [], 0, 0
            while ia < len(a) or ib < len(b):
                if ia < len(a):
                    out.append(a[ia]); ia += 1
                if ib < len(b):
                    out.append(b[ib]); ib += 1
            return out

        # ---- prologue: batch 0's Q/K, dc-major so the four PSUM
        # accumulation groups chase the arriving hT pieces concurrently.
        pro_specs = [(wq_sb, bq_sb, qT_sb, 0), (wq_sb, bq_sb, qT_sb, 1),
                     (wk_sb, bk_sb, kT_sb, 0), (wk_sb, bk_sb, kT_sb, 1)]
        pro_tiles = [ps_sc.tile([P, HPC, CH], f32, tag="sc", name="pro_ps")
                     for _ in range(2)]
        for dc in range(DCH):
            for g, (w_sb_, b_sb_, dst_, ci) in enumerate(pro_specs):
                nc.tensor.matmul(
                    pro_tiles[g // 2][:, g % 2, :], w_sb_[:, dc, :],
                    hT_sb[:, dc, ci * CH:(ci + 1) * CH],
                    start=(dc == 0), stop=(dc == DCH - 1))
        # c0 adds on DVE, c1 adds on ACT: batch 0's first score pair only
        # needs the c0 halves, so both engines converge on it fast.
        for g, (w_sb_, b_sb_, dst_, ci) in enumerate(pro_specs):
            osl = slice(ci * CH, (ci + 1) * CH)
            if ci % 2 == 0:
                nc.vector.tensor_scalar_add(
                    dst_[:, osl], pro_tiles[g // 2][:, g % 2, :], b_sb_[:])
            else:
                nc.scalar.activation(dst_[:, osl],
                                     pro_tiles[g // 2][:, g % 2, :],
                                     AF.Identity, bias=b_sb_[:])

        # ---- per-window PE work queues (flexible 128-mode thunks).
        # Q/K of later batches are pulled into earlier, ACT-slack windows;
        # thunks gated by late DMA arrivals sit at the tail of a window.
        prev = None          # (bi, pps, o_sb) of previous batch
        all_pps = []
        for bi in range(B):
            pps = []         # pps[uc][c2] = [128, 2, 512] bf16 probs tile
            o_sb = o_pool.tile([P, HPC, TT, DH], f32, name="o_sb")
            if bi == 0:
                queue = interleave(v_thunks(0), qk_thunks(1))
            elif bi < B - 1:
                queue = interleave(av_thunks(prev[0], prev[1], prev[2]),
                                   v_thunks(bi) + qk_thunks(bi + 1, nsub=4))
            else:
                queue = interleave(av_thunks(prev[0], prev[1], prev[2]),
                                   v_thunks(bi))
            total = sum(c for c, _ in queue)
            spent = 0.0
            for uc in range(DCH):
                usl = slice(bi * S + uc * P, bi * S + (uc + 1) * P)
                cpps = []
                for c2 in range(2):
                    qsl = slice(bi * S + c2 * CH, bi * S + (c2 + 1) * CH)
                    sc = ps_sc.tile([P, HPC, CH], f32, tag="sc", name="sc_ps")
                    for hl in range(HPC):
                        hsl = slice(hl * DH, (hl + 1) * DH)
                        nc.tensor.matmul(
                            sc[:, hl, :], kT_sb[hsl, usl], qT_sb[hsl, qsl],
                            start=True, stop=True)
                    pp = probs_pool.tile([P, HPC, CH], bf16, name="pp")
                    nc.scalar.activation(
                        pp[:], sc[:], AF.Exp,
                        bias=mask_bias(uc, bi), scale=scale)
                    cpps.append(pp)
                pps.append(cpps)
                target = total * (uc + 1) / DCH
                while queue and (spent < target or uc == DCH - 1):
                    c, th = queue.pop(0)
                    th()
                    spent += c
            if prev is not None:
                nc.sync.dma_start(out_d.ap()[prev[0]], prev[2][:])
            prev = (bi, pps, o_sb)
        # epilogue: last batch's attention output, head-major with the two
        # free PSUM pools alternating so consecutive AV groups double-buffer;
        # each head's half-DMA starts as soon as its groups are normalized.
        epi = av_thunks(prev[0], prev[1], prev[2], pools=(ps_av, ps_mm),
                        order="hl", mul_on_act=True)
        for i, (c, th) in enumerate(epi):
            th()
            if i == TT - 1:
                nc.sync.dma_start(out_d.ap()[prev[0]][:, 0], prev[2][:, 0])
        nc.sync.dma_start(out_d.ap()[prev[0]][:, 1], prev[2][:, 1])

    nc.compile()
    return nc


def _get_nc():
    global _compiled_nc
    if _compiled_nc is None:
        _compiled_nc = _build()
    return _compiled_nc


def prepare_in_maps(hidden_states, attention_mask, Wq, bq, Wk, bk, Wv, bv):
    bf16 = ml_dtypes.bfloat16

    hs = np.asarray(hidden_states, dtype=np.float32)            # [S, B, D]
    hT = np.ascontiguousarray(hs.transpose(2, 1, 0).reshape(D, BS)).astype(bf16)
    maskT = np.ascontiguousarray(
        np.asarray(attention_mask, dtype=np.float32).reshape(B, S).T)
    Wq = np.asarray(Wq, dtype=np.float32)
    Wk = np.asarray(Wk, dtype=np.float32)
    Wv = np.asarray(Wv, dtype=np.float32)
    bq = np.asarray(bq, dtype=np.float32)
    bk = np.asarray(bk, dtype=np.float32)
    bv = np.asarray(bv, dtype=np.float32)

    def swizzle(w, sl):
        # [p, dc, m] layout so the device DMA reads 2KB-contiguous lines
        wT = w[sl, :].T.reshape(DCH, P, P).transpose(1, 0, 2)
        return np.ascontiguousarray(wT.reshape(P, DCH * P)).astype(bf16)

    # maskT packed as [p, uc, b] -> [128, 32]
    mask_pk = maskT.reshape(DCH, P, B).transpose(1, 0, 2).reshape(P, DCH * B)
    in_maps = []
    for c in range(NCORES):
        sl = slice(P * c, P * (c + 1))
        misc = np.empty((P, 2 + P + DCH * B), dtype=np.float32)
        misc[:, 0] = bq[sl]
        misc[:, 1] = bk[sl]
        misc[:, 2:2 + P] = np.broadcast_to(bv[sl][None, :], (P, P))
        misc[:, 2 + P:] = mask_pk
        in_maps.append({
            "hT": hT,
            "wq": swizzle(Wq, sl),
            "wk": swizzle(Wk, sl),
            "wv": swizzle(Wv, sl),
            "misc": misc,
        })
    return in_maps


def kernel(hidden_states, attention_mask, Wq, bq, Wk, bk, Wv, bv):
    global last_exec_time_ns, last_results
    from concourse.bass_utils import run_bass_kernel_spmd

    nc = _get_nc()
    in_maps = prepare_in_maps(hidden_states, attention_mask,
                              Wq, bq, Wk, bk, Wv, bv)

    trace = bool(int(os.environ.get("KERNEL_TRACE", "0")))
    tmpdir = os.environ.get("KERNEL_TRACE_DIR") or None
    res = run_bass_kernel_spmd(nc, in_maps, core_ids=list(range(NCORES)),
                               trace=trace, tmpdir=tmpdir)
    last_exec_time_ns = res.exec_time_ns
    last_results = res

    # gather: per-core out [B, P, HPC, TT, DH] -> full [S, B, D]
    outs = np.stack([np.asarray(res.results[c]["out"]) for c in range(NCORES)],
                    axis=0)                             # [C, B, p, hl, tt, j]
    # s = tt*128 + p ; d = c*128 + hl*64 + j
    full = outs.transpose(4, 2, 1, 0, 3, 5).reshape(S, B, D)
    return np.ascontiguousarray(full.astype(np.float32))


# revision 13
# speedup vs baseline: 1.1376x; 1.0508x over previous
"""BertSelfAttention Trainium2 kernel.

Shapes: hidden_states [S=1024, B=4, D=1024], H=16 heads of DH=64.
Sharding: 2 heads per core (8 cores). Each core receives the full hidden
states (pre-transposed + bf16-cast on host) and a 128-row slice of each
projection weight, computes the full attention chain for its two heads with
no cross-core communication.

Device-side layout tricks:
  - scores are computed transposed (scoresT[u, t] = q_t . k_u) so the
    additive attention mask (per key position u) is a per-partition bias
    that fuses into the Exp activation: probsT = exp(scores/8 + mask).
    Both heads' K=64 score matmuls are row-tiled into the PE array
    concurrently (row_grp 0 / 64) and write one shared [128, 2, 512] PSUM
    tile so the pair always issues back-to-back.
  - AV runs in out=[t, j] orientation: lhsT = probsT chunk (K=128, M=128
    query positions), rhs = V tile with a prepended ones-column (N=65), so
    ctx rows land on partitions and the softmax denominator lands in
    column 0. Normalization is then a per-partition reciprocal + scalar
    multiply on the vector engine (no cross-partition broadcast needed).
  - software pipeline: scores/exp paced by the scalar engine; AV of the
    previous batch, V projection and Q/K projection of later batches woven
    into the PE stream by a cost-paced queue so every batch window carries
    an even PE load.
"""

import os
import numpy as np
import ml_dtypes

S, B, D, H = 1024, 4, 1024, 16
DH = D // H          # 64
NCORES = 8
HPC = H // NCORES    # heads per core = 2
P = 128              # partitions / d-tile / t-tile
DCH = D // P         # 8 contraction tiles
BS = B * S           # 4096 flattened (b, s)
CH = 512             # matmul free-dim chunk (fp32 rhs limit)
TT = S // P          # 8 query tiles per batch

_compiled_nc = None
last_exec_time_ns = None
last_results = None


def _build():
    import concourse.bacc as bacc
    import concourse.mybir as mybir
    import concourse.tile as tile
    from contextlib import ExitStack

    f32 = mybir.dt.float32
    bf16 = mybir.dt.bfloat16
    AF = mybir.ActivationFunctionType

    nc = bacc.Bacc("TRN2", target_bir_lowering=False, debug=False,
                   num_devices=NCORES)

    hT_d = nc.dram_tensor("hT", [D, BS], bf16, kind="ExternalInput")
    # weights pre-swizzled on host to [p, dc, m] so the DMA moves 2KB lines
    wq_d = nc.dram_tensor("wq", [P, DCH * P], bf16, kind="ExternalInput")
    wk_d = nc.dram_tensor("wk", [P, DCH * P], bf16, kind="ExternalInput")
    wv_d = nc.dram_tensor("wv", [P, DCH * P], bf16, kind="ExternalInput")
    # packed per-partition constants: [bq | bk | bvb(128) | maskT(8*4)]
    misc_d = nc.dram_tensor("misc", [P, 2 + P + DCH * B], f32,
                            kind="ExternalInput")
    # out[b, p, hl, tt, j]: query position s = tt*128 + p, feature = hl*64+j
    out_d = nc.dram_tensor("out", [B, P, HPC, TT, DH], f32,
                           kind="ExternalOutput")

    with tile.TileContext(nc) as tc, ExitStack() as ctx:
        persist = ctx.enter_context(tc.tile_pool(name="persist", bufs=1))
        probs_pool = ctx.enter_context(tc.tile_pool(name="probs", bufs=36))
        small = ctx.enter_context(tc.tile_pool(name="small", bufs=4))
        o_pool = ctx.enter_context(tc.tile_pool(name="outp", bufs=2))
        # PSUM budget (8 banks): scores 2x[128,2,512] (4) + proj 2 + av 2
        ps_sc = ctx.enter_context(tc.tile_pool(name="ps_sc", bufs=2, space="PSUM"))
        ps_mm = ctx.enter_context(tc.tile_pool(name="ps_mm", bufs=2, space="PSUM"))
        ps_av = ctx.enter_context(tc.tile_pool(name="ps_av", bufs=2, space="PSUM"))

        # ---- persistent SBUF tensors ----
        hT_sb = persist.tile([P, DCH, BS], bf16)        # hidden^T, d-tiled
        wq_sb = persist.tile([P, DCH, P], bf16)
        wk_sb = persist.tile([P, DCH, P], bf16)
        wv_sb = persist.tile([P, DCH, P], bf16)
        misc_sb = persist.tile([P, 2 + P + DCH * B], f32)
        qT_sb = persist.tile([P, BS], bf16)             # Q^T [i, t]
        kT_sb = persist.tile([P, BS], bf16)             # K^T [i, t]
        # V in [t, j] layout + ones column per head: [t-part, t-tile, head, DH+1]
        v_sb = persist.tile([P, BS // P, HPC, DH + 1], bf16)
        dummy_sb = persist.tile([P, CH], bf16)

        bq_sb = misc_sb[:, 0:1]
        bk_sb = misc_sb[:, 1:2]
        bvb_sb = misc_sb[:, 2:2 + P]

        def mask_bias(uc, bi):
            c = 2 + P + uc * B + bi
            return misc_sb[:, c:c + 1]

        # ---- input DMAs ----
        # wq/wk first (contiguous, host-swizzled), then batch 0's hT pieces
        # alternating between the SP and ACT HWDGE queues, then wv/misc,
        # batch 1 split across both queues, batches 2-3 on SP.
        hT_re = hT_d.ap().rearrange("(dc p) t -> p dc t", p=P)

        def hT_piece(q, dc, eng):
            qsl = slice(q * S, (q + 1) * S)
            eng.dma_start(hT_sb[:, dc, qsl], hT_re[:, dc, qsl])

        nc.gpsimd.memset(dummy_sb[:], 0.0)
        nc.sync.dma_start(wq_sb[:], wq_d.ap())
        nc.scalar.dma_start(wk_sb[:], wk_d.ap())
        hT_piece(0, 0, nc.sync)
        hT_piece(0, 1, nc.scalar)
        nc.scalar.dma_start(wv_sb[:], wv_d.ap())
        hT_piece(0, 2, nc.sync)
        hT_piece(0, 3, nc.scalar)
        nc.scalar.dma_start(misc_sb[:], misc_d.ap())
        for dc in range(4, DCH):
            hT_piece(0, dc, nc.sync if dc % 2 == 0 else nc.scalar)
        # batch 1 even pieces early on SP; odd pieces are emitted inside
        # batch 0's uc loop so their ACT-queue issue slices (which block
        # in-stream on DGE ring occupancy) land after the first Exps.
        for dc in range(0, DCH, 2):
            hT_piece(1, dc, nc.sync)
        # batches 2-3 on SP in two bulk DMAs each, naturally serialized
        # behind batches 0-1 so they don't steal HBM bandwidth early.
        for q in range(2, B):
            for half in range(2):
                dsl = slice(half * 4, half * 4 + 4)
                qsl = slice(q * S, (q + 1) * S)
                nc.sync.dma_start(hT_sb[:, dsl, qsl], hT_re[:, dsl, qsl])

        nc.vector.memset(v_sb[:, :, :, 0:1], 1.0)

        # HAM warmup: dead matmuls ramp the PE clock while inputs load.
        for _ in range(8):
            d_ps = ps_av.tile([P, CH], f32, tag="av", name="d_ps")
            nc.tensor.matmul(d_ps[:], dummy_sb[:, 0:P], dummy_sb[:],
                             start=True, stop=True)

        scale = 1.0 / float(np.sqrt(DH))

        # ---- thunks for the 128-mode PE work queue (cost, fn) ----
        def emit_qk_chunk(w_sb, b_sb, dst, ci):
            sl = slice(ci * CH, (ci + 1) * CH)
            qk_ps = ps_mm.tile([P, CH], f32, tag="mm", name="qk_ps")
            for dc in range(DCH):
                nc.tensor.matmul(
                    qk_ps[:], w_sb[:, dc, :], hT_sb[:, dc, sl],
                    start=(dc == 0), stop=(dc == DCH - 1))
            nc.vector.tensor_scalar_add(dst[:, sl], qk_ps[:], b_sb[:])

        def emit_v_tile(tt):
            tsl = slice(tt * P, (tt + 1) * P)
            v_ps = ps_mm.tile([P, CH], f32, tag="mm", name="v_ps")
            for dc in range(DCH):
                nc.tensor.matmul(
                    v_ps[:, 0:P], hT_sb[:, dc, tsl], wv_sb[:, dc, :],
                    start=(dc == 0), stop=(dc == DCH - 1))
            nc.vector.tensor_add(
                v_sb[:, tt, 0:HPC, 1:DH + 1],
                v_ps[:, 0:P].rearrange("p (h j) -> p h j", j=DH),
                bvb_sb[:].rearrange("p (h j) -> p h j", j=DH))

        def emit_av_tile(bi, hl, tt, pps, o_sb, pool=None, mul_on_act=False):
            # ctx^T tile: out[t, 0] = softmax denominator, out[t, 1:] = ctx
            p_ = pool or ps_av
            av_ps = p_.tile([P, CH], f32, tag="mm" if p_ is ps_mm else "av",
                            name="av_ps")
            av = av_ps[:, 0:DH + 1]
            c2, t0 = tt // 4, (tt % 4) * P
            for uc in range(DCH):
                nc.tensor.matmul(
                    av, pps[uc][c2][:, hl, t0:t0 + P],
                    v_sb[:, bi * TT + uc, hl, :],
                    start=(uc == 0), stop=(uc == DCH - 1))
            rcp = small.tile([P, 1], f32, name="rcp")
            nc.vector.reciprocal_approx_fast(rcp[:], av_ps[:, 0:1])
            if mul_on_act:
                nc.scalar.activation(o_sb[:, hl, tt, :], av_ps[:, 1:DH + 1],
                                     AF.Copy, scale=rcp[:])
            else:
                nc.vector.tensor_scalar_mul(
                    o_sb[:, hl, tt, :], av_ps[:, 1:DH + 1], rcp[:])

        def qk_chunk_subthunks(w_sb, b_sb, dst, ci, nsub, min_uc=0):
            st = {}
            step = DCH // nsub

            def sub(lo):
                def fn():
                    sl = slice(ci * CH, (ci + 1) * CH)
                    if lo == 0:
                        st["ps"] = ps_mm.tile([P, CH], f32, tag="mm",
                                              name="qk_ps")
                    for dc in range(lo, lo + step):
                        nc.tensor.matmul(
                            st["ps"][:], w_sb[:, dc, :], hT_sb[:, dc, sl],
                            start=(dc == 0), stop=(dc == DCH - 1))
                    if lo + step == DCH:
                        nc.vector.tensor_scalar_add(dst[:, sl], st["ps"][:],
                                                    b_sb[:])
                return (1.72 / nsub, min_uc, fn)
            return [sub(i * step) for i in range(nsub)]

        def qk_thunks(bi, nsub=1, min_uc=0):
            return [th for (w_sb_, b_sb_, dst_) in ((wq_sb, bq_sb, qT_sb),
                                                    (wk_sb, bk_sb, kT_sb))
                    for ci in (2 * bi, 2 * bi + 1)
                    for th in qk_chunk_subthunks(w_sb_, b_sb_, dst_, ci,
                                                    nsub, min_uc)]

        def v_thunks(bi):
            return [(0.45, 0, lambda t=tt: emit_v_tile(t))
                    for tt in range(TT * bi, TT * bi + TT)]

        def av_thunks(bi, pps, o_sb, pools=(None,), order="tt",
                      mul_on_act=False):
            idx = ([(t, h) for t in range(TT) for h in range(HPC)]
                   if order == "tt" else
                   [(t, h) for h in range(HPC) for t in range(TT)])
            return [(0.33, 0, lambda h=hl, t=tt, p=pools[i % len(pools)]:
                     emit_av_tile(bi, h, t, pps, o_sb, p, mul_on_act))
                    for i, (tt, hl) in enumerate(idx)]

        def interleave(a, b):
            out, ia, ib = [], 0, 0
            while ia < len(a) or ib < len(b):
                if ia < len(a):
                    out.append(a[ia]); ia += 1
                if ib < len(b):
                    out.append(b[ib]); ib += 1
            return out

        # ---- prologue: batch 0's Q/K, dc-major so the four PSUM
        # accumulation groups chase the arriving hT pieces concurrently.
        pro_specs = [(wq_sb, bq_sb, qT_sb, 0), (wq_sb, bq_sb, qT_sb, 1),
                     (wk_sb, bk_sb, kT_sb, 0), (wk_sb, bk_sb, kT_sb, 1)]
        pro_tiles = [ps_sc.tile([P, HPC, CH], f32, tag="sc", name="pro_ps")
                     for _ in range(2)]
        for dc in range(DCH):
            for g, (w_sb_, b_sb_, dst_, ci) in enumerate(pro_specs):
                nc.tensor.matmul(
                    pro_tiles[g // 2][:, g % 2, :], w_sb_[:, dc, :],
                    hT_sb[:, dc, ci * CH:(ci + 1) * CH],
                    start=(dc == 0), stop=(dc == DCH - 1))
        # c0 adds on DVE, c1 adds on ACT: batch 0's first score pair only
        # needs the c0 halves, so both engines converge on it fast.
        for g, (w_sb_, b_sb_, dst_, ci) in enumerate(pro_specs):
            osl = slice(ci * CH, (ci + 1) * CH)
            if ci % 2 == 0:
                nc.vector.tensor_scalar_add(
                    dst_[:, osl], pro_tiles[g // 2][:, g % 2, :], b_sb_[:])
            else:
                nc.scalar.activation(dst_[:, osl],
                                     pro_tiles[g // 2][:, g % 2, :],
                                     AF.Identity, bias=b_sb_[:])

        # ---- per-window PE work queues (flexible 128-mode thunks).
        # Q/K of later batches are pulled into earlier, ACT-slack windows;
        # thunks gated by late DMA arrivals sit at the tail of a window.
        prev = None          # (bi, pps, o_sb) of previous batch
        all_pps = []
        for bi in range(B):
            pps = []         # pps[uc][c2] = [128, 2, 512] bf16 probs tile
            o_sb = o_pool.tile([P, HPC, TT, DH], f32, name="o_sb")
            if bi == 0:
                queue = v_thunks(0) + qk_thunks(1, min_uc=4)
            elif bi < B - 1:
                queue = interleave(av_thunks(prev[0], prev[1], prev[2]),
                                   v_thunks(bi) + qk_thunks(bi + 1, nsub=4,
                                                            min_uc=1))
            else:
                queue = interleave(av_thunks(prev[0], prev[1], prev[2]),
                                   v_thunks(bi))
            total = sum(c for c, _mu, _ in queue)
            spent = 0.0
            for uc in range(DCH):
                usl = slice(bi * S + uc * P, bi * S + (uc + 1) * P)
                cpps = []
                for c2 in range(2):
                    qsl = slice(bi * S + c2 * CH, bi * S + (c2 + 1) * CH)
                    sc = ps_sc.tile([P, HPC, CH], f32, tag="sc", name="sc_ps")
                    for hl in range(HPC):
                        hsl = slice(hl * DH, (hl + 1) * DH)
                        nc.tensor.matmul(
                            sc[:, hl, :], kT_sb[hsl, usl], qT_sb[hsl, qsl],
                            start=True, stop=True)
                    pp = probs_pool.tile([P, HPC, CH], bf16, name="pp")
                    nc.scalar.activation(
                        pp[:], sc[:], AF.Exp,
                        bias=mask_bias(uc, bi), scale=scale)
                    cpps.append(pp)
                pps.append(cpps)
                if bi == 0 and uc < 4:
                    hT_piece(1, 2 * uc + 1, nc.scalar)
                target = total * (uc + 1) / DCH
                while queue and queue[0][1] <= uc and (spent < target
                                                       or uc == DCH - 1):
                    c, _mu, th = queue.pop(0)
                    th()
                    spent += c
            if prev is not None:
                nc.sync.dma_start(out_d.ap()[prev[0]], prev[2][:])
            prev = (bi, pps, o_sb)
        # epilogue: last batch's attention output, head-major with the two
        # free PSUM pools alternating so consecutive AV groups double-buffer;
        # each head's half-DMA starts as soon as its groups are normalized.
        epi = av_thunks(prev[0], prev[1], prev[2], pools=(ps_av, ps_mm),
                        order="hl", mul_on_act=True)
        for i, (c, _mu, th) in enumerate(epi):
            th()
            if i == TT - 1:
                nc.sync.dma_start(out_d.ap()[prev[0]][:, 0], prev[2][:, 0])
        nc.sync.dma_start(out_d.ap()[prev[0]][:, 1], prev[2][:, 1])

    nc.compile()
    return nc


def _get_nc():
    global _compiled_nc
    if _compiled_nc is None:
        _compiled_nc = _build()
    return _compiled_nc


def prepare_in_maps(hidden_states, attention_mask, Wq, bq, Wk, bk, Wv, bv):
    bf16 = ml_dtypes.bfloat16

    hs = np.asarray(hidden_states, dtype=np.float32)            # [S, B, D]
    hT = np.ascontiguousarray(hs.transpose(2, 1, 0).reshape(D, BS)).astype(bf16)
    maskT = np.ascontiguousarray(
        np.asarray(attention_mask, dtype=np.float32).reshape(B, S).T)
    Wq = np.asarray(Wq, dtype=np.float32)
    Wk = np.asarray(Wk, dtype=np.float32)
    Wv = np.asarray(Wv, dtype=np.float32)
    bq = np.asarray(bq, dtype=np.float32)
    bk = np.asarray(bk, dtype=np.float32)
    bv = np.asarray(bv, dtype=np.float32)

    def swizzle(w, sl):
        # [p, dc, m] layout so the device DMA reads 2KB-contiguous lines
        wT = w[sl, :].T.reshape(DCH, P, P).transpose(1, 0, 2)
        return np.ascontiguousarray(wT.reshape(P, DCH * P)).astype(bf16)

    # maskT packed as [p, uc, b] -> [128, 32]
    mask_pk = maskT.reshape(DCH, P, B).transpose(1, 0, 2).reshape(P, DCH * B)
    in_maps = []
    for c in range(NCORES):
        sl = slice(P * c, P * (c + 1))
        misc = np.empty((P, 2 + P + DCH * B), dtype=np.float32)
        misc[:, 0] = bq[sl]
        misc[:, 1] = bk[sl]
        misc[:, 2:2 + P] = np.broadcast_to(bv[sl][None, :], (P, P))
        misc[:, 2 + P:] = mask_pk
        in_maps.append({
            "hT": hT,
            "wq": swizzle(Wq, sl),
            "wk": swizzle(Wk, sl),
            "wv": swizzle(Wv, sl),
            "misc": misc,
        })
    return in_maps


def kernel(hidden_states, attention_mask, Wq, bq, Wk, bk, Wv, bv):
    global last_exec_time_ns, last_results
    from concourse.bass_utils import run_bass_kernel_spmd

    nc = _get_nc()
    in_maps = prepare_in_maps(hidden_states, attention_mask,
                              Wq, bq, Wk, bk, Wv, bv)

    trace = bool(int(os.environ.get("KERNEL_TRACE", "0")))
    tmpdir = os.environ.get("KERNEL_TRACE_DIR") or None
    res = run_bass_kernel_spmd(nc, in_maps, core_ids=list(range(NCORES)),
                               trace=trace, tmpdir=tmpdir)
    last_exec_time_ns = res.exec_time_ns
    last_results = res

    # gather: per-core out [B, P, HPC, TT, DH] -> full [S, B, D]
    outs = np.stack([np.asarray(res.results[c]["out"]) for c in range(NCORES)],
                    axis=0)                             # [C, B, p, hl, tt, j]
    # s = tt*128 + p ; d = c*128 + hl*64 + j
    full = outs.transpose(4, 2, 1, 0, 3, 5).reshape(S, B, D)
    return np.ascontiguousarray(full.astype(np.float32))
